# revision 4
# baseline (speedup 1.0000x reference)
"""Trainium2 Bass kernel for ConvMultiHeadAttention.

Reference computation (per batch element b):
  q/k/v projections: conv1d(25 taps along d_model=512, VALID) -> (512, 488, 16)
                     avgpool(8 along H) -> (512, 61, 16)
                     flat-reshape -> (S=976, 512) viewed as (S, 8 heads, 64)
  attention: scores = Q K^T / 8, attn = softmax, x = attn V
  outputs: x (B, 976, 512) scaled by w_out + b_out, and attn (B, 8, 976, 976)

Sharding: data-parallel over batch B=8, one batch element per NeuronCore.

Device algorithm per core:
  - conv+pool folded into a stride-8, width-32 conv == matmul W'(512,32) @ P(32,976)
    where P are strided patches of the input image (affine DMA gather).
  - conv output Y(512,976) is written flat to DRAM scratch; the reference's
    reshape (512*976,) -> (976, 512) is a free reinterpretation of that buffer.
  - Q,K are reloaded row-wise and PE-transposed into (d=64, s=976) per-head
    layout; V is used row-wise (s, d) directly.
  - pass A (per head, per 128-row q chunk): scores = QhT.T @ KhT in PSUM,
    ACT computes exp(s/8) with fused per-row accumulation (softmax denom),
    DVE normalizes, DMA writes attn rows.
  - pass B: transposed scores K-major, exp'd, then AV matmul accumulates
    outT(64, 976) over k chunks; PE-transpose + per-row scale gives x.

Matmul dtype: bf16 operands with fp32 PSUM accumulation (the fp32/f32r
matmul paths split the 4-byte weight load into two LW sub-instructions and
hit walrus's sync-wait-count limit under Tile). Scores feed a softmax over
~1000 near-uniform logits, so bf16 logit error is far below tolerance; the
single bf16 rounding of V/exp(s) costs ~1e-3 relative on x.
"""

import numpy as np

import concourse.bacc as bacc
import concourse.bass as bass
import concourse.tile as tile
from concourse import mybir
from concourse.bass_utils import run_bass_kernel_spmd
from concourse.masks import make_identity

F32 = mybir.dt.float32
BF16 = mybir.dt.bfloat16
AF = mybir.ActivationFunctionType
ALU = mybir.AluOpType

B = 8
NH, DH = 8, 64
S = 976          # sequence positions after conv+pool reshape
C = 512          # channels = NH*DH
DM, L = 512, 16  # conv input "image": d_model x max_len
KW = 32          # folded conv width (25-tap conv + 8-wide avgpool, stride 8)
H2 = 61          # pooled positions, S = H2 * L

NSPLITS = ((0, 512), (512, S))                        # PSUM-bank splits of 976
SCH = [(i * 128, min(128, S - i * 128)) for i in range(8)]  # 7x128 + 80

TRACE = False
TRACE_KW: dict = {}
LAST_RESULTS = None


def _build_nc():
    nc = bacc.Bacc("TRN2", target_bir_lowering=False, debug=False)

    xin = {t: nc.dram_tensor(f"x{t}", [DM, L], F32, kind="ExternalInput")
           for t in "qkv"}
    win = {t: nc.dram_tensor(f"w{t}T", [KW, C], F32, kind="ExternalInput")
           for t in "qkv"}
    bout = nc.dram_tensor("bout", [1, 1], F32, kind="ExternalInput")
    attn_o = nc.dram_tensor("attn_o", [NH, S, S], F32, kind="ExternalOutput")
    x_o = nc.dram_tensor("x_o", [S, C], F32, kind="ExternalOutput")
    # conv outputs, stored flat in bf16
    y = {t: nc.dram_tensor(f"y{t}", [C * S], BF16, kind="Internal")
         for t in "qkv"}

    with tile.TileContext(nc) as tc:
        _body(tc, xin, win, bout, attn_o, x_o, y)
    nc.compile()
    return nc


def _body(tc, xin, win, bout, attn_o, x_o, y):
    nc = tc.nc
    import contextlib
    ctx = contextlib.ExitStack()
    with ctx:
        consts = ctx.enter_context(tc.tile_pool(name="consts", bufs=1))
        persist = ctx.enter_context(tc.tile_pool(name="persist", bufs=1))
        stage = ctx.enter_context(tc.tile_pool(name="stage", bufs=3))
        work = ctx.enter_context(tc.tile_pool(name="work", bufs=2))
        mm_psum = ctx.enter_context(tc.tile_pool(name="mm_psum", bufs=2, space="PSUM"))
        tp_psum = ctx.enter_context(tc.tile_pool(name="tp_psum", bufs=2, space="PSUM"))
        acc_psum = ctx.enter_context(tc.tile_pool(name="acc_psum", bufs=1, space="PSUM"))

        ident = consts.tile([128, 128], F32, tag="ident")
        make_identity(nc, ident)
        ident_h = consts.tile([128, 128], BF16, tag="ident_h")
        make_identity(nc, ident_h)

        bb = consts.tile([128, 1], F32, tag="bb")
        bsrc = bass.AP(tensor=bout.ap().tensor, offset=0, ap=[[0, 128], [1, 1]])
        nc.gpsimd.dma_start(out=bb, in_=bsrc)

        # ---- phase 1: conv projections -> DRAM scratch (flat, f32r) ----
        for t in "qkv":
            p_t = consts.tile([KW, H2, L], F32, tag=f"patch{t}", name=f"patch{t}")
            # P[k', h2, l] = x[8*h2 + k', l]
            psrc = bass.AP(tensor=xin[t].ap().tensor, offset=0,
                           ap=[[L, KW], [8 * L, H2], [1, L]])
            nc.sync.dma_start(out=p_t, in_=psrc)
            w_t = consts.tile([KW, C], F32, tag=f"w{t}", name=f"w{t}")
            nc.sync.dma_start(out=w_t, in_=win[t].ap())
            # round conv operands to bf16 for the fast matmul path
            p_r = consts.tile([KW, H2 * L], BF16, tag=f"pr{t}", name=f"pr{t}")
            nc.vector.tensor_copy(p_r, p_t.rearrange("k a b -> k (a b)"))
            w_r = consts.tile([KW, C], BF16, tag=f"wr{t}", name=f"wr{t}")
            nc.vector.tensor_copy(w_r, w_t)

            ywview = y[t].ap().rearrange("(c u) -> c u", u=S)
            for ci in range(4):
                ps = mm_psum.tile([128, S], F32, tag="mm")
                for (n0, n1) in NSPLITS:
                    nc.tensor.matmul(
                        ps[:, n0:n1],
                        lhsT=w_r[:, ci * 128:(ci + 1) * 128],
                        rhs=p_r[:, n0:n1],
                        start=True, stop=True)
                y_sb = stage.tile([128, S], BF16, tag="ysb")
                nc.scalar.copy(y_sb, ps)
                nc.sync.dma_start(out=ywview[ci * 128:(ci + 1) * 128, :], in_=y_sb)

        # ---- phase 2: per-head transposed Q,K (f32r); row-major V (f32r) ----
        qT = [persist.tile([128, S], BF16, tag=f"qT{j}", name=f"qT{j}") for j in range(4)]
        kT = [persist.tile([128, S], BF16, tag=f"kT{j}", name=f"kT{j}") for j in range(4)]
        for t, dst in (("q", qT), ("k", kT)):
            sview = y[t].ap().rearrange("(s d) -> s d", d=C)
            for (s0, sz) in SCH:
                rows = stage.tile([128, C], BF16, tag="rows")
                nc.sync.dma_start(out=rows[:sz], in_=sview[s0:s0 + sz, :])
                for j in range(4):
                    pst = tp_psum.tile([128, 128], BF16, tag="tp")
                    nc.tensor.transpose(pst[:, :sz],
                                        rows[:sz, j * 128:(j + 1) * 128],
                                        ident_h[:sz, :sz])
                    nc.vector.tensor_copy(dst[j][:, s0:s0 + sz], pst[:, :sz])

        v_rows = [persist.tile([128, C], BF16, tag=f"vr{sc}", name=f"vr{sc}")
                  for sc in range(8)]
        vview = y["v"].ap().rearrange("(s d) -> s d", d=C)
        for sc, (s0, sz) in enumerate(SCH):
            nc.sync.dma_start(out=v_rows[sc][:sz], in_=vview[s0:s0 + sz, :])

        x_sb = [persist.tile([128, C], F32, tag=f"xsb{sc}", name=f"xsb{sc}")
                for sc in range(8)]

        # ---- phase 3: attention per head ----
        for h in range(NH):
            j, off = h // 2, (h % 2) * 64
            qTh = qT[j][off:off + 64, :]
            kTh = kT[j][off:off + 64, :]
            R_all = work.tile([128, 8], F32, tag="rall", name=f"rall{h}")

            # pass A: q-major scores -> exp(+denom) -> normalize -> attn out
            for qc, (s0, sz) in enumerate(SCH):
                sA = mm_psum.tile([128, S], F32, tag="mm")
                for (n0, n1) in NSPLITS:
                    nc.tensor.matmul(sA[:sz, n0:n1],
                                     lhsT=qTh[:, s0:s0 + sz],
                                     rhs=kTh[:, n0:n1],
                                     start=True, stop=True)
                e = work.tile([128, S], F32, tag="e")
                d = work.tile([128, 1], F32, tag="d")
                nc.scalar.activation(e[:sz], sA[:sz], AF.Exp, scale=0.125,
                                     accum_out=d[:sz])
                nc.vector.reciprocal(R_all[:sz, qc:qc + 1], d[:sz])
                a = work.tile([128, S], F32, tag="a")
                nc.vector.tensor_scalar_mul(a[:sz], e[:sz], R_all[:sz, qc:qc + 1])
                nc.sync.dma_start(out=attn_o.ap()[h, s0:s0 + sz, :], in_=a[:sz])

            # pass B: k-major scores -> exp (f32r) -> AV accumulate
            oT0 = acc_psum.tile([64, 512], F32, tag="oT0", name=f"oT0_{h}")
            oT1 = acc_psum.tile([64, S - 512], F32, tag="oT1", name=f"oT1_{h}")
            for kc, (s0, sz) in enumerate(SCH):
                sB = mm_psum.tile([128, S], F32, tag="mm")
                for (n0, n1) in NSPLITS:
                    nc.tensor.matmul(sB[:sz, n0:n1],
                                     lhsT=kTh[:, s0:s0 + sz],
                                     rhs=qTh[:, n0:n1],
                                     start=True, stop=True)
                eT = work.tile([128, S], BF16, tag="et")
                nc.scalar.activation(eT[:sz], sB[:sz], AF.Exp, scale=0.125)
                vh = v_rows[kc][:sz, h * DH:(h + 1) * DH]
                nc.tensor.matmul(oT0, lhsT=vh, rhs=eT[:sz, 0:512],
                                 start=(kc == 0), stop=(kc == 7))
                nc.tensor.matmul(oT1, lhsT=vh, rhs=eT[:sz, 512:S],
                                 start=(kc == 0), stop=(kc == 7))

            o_sb = work.tile([64, S], F32, tag="osb")
            nc.vector.tensor_copy(o_sb[:, 0:512], oT0)
            nc.vector.tensor_copy(o_sb[:, 512:S], oT1)
            for qc, (s0, sz) in enumerate(SCH):
                pst = tp_psum.tile([128, 128], F32, tag="tp")
                nc.tensor.transpose(pst[:sz, :DH], o_sb[:, s0:s0 + sz],
                                    ident[:64, :64])
                nc.vector.tensor_scalar(
                    out=x_sb[qc][:sz, h * DH:(h + 1) * DH],
                    in0=pst[:sz, :DH],
                    scalar1=R_all[:sz, qc:qc + 1],
                    scalar2=bb[:sz],
                    op0=ALU.mult, op1=ALU.add)

        for sc, (s0, sz) in enumerate(SCH):
            nc.sync.dma_start(out=x_o.ap()[s0:s0 + sz, :], in_=x_sb[sc][:sz])


def _fold_pool_into_conv(w):
    """(C,1,25,1) conv weight + 8-wide avgpool -> (C, 32) stride-8 weights."""
    w2 = np.asarray(w, np.float64)[:, 0, :, 0]          # (C, 25)
    out = np.zeros((w2.shape[0], KW), np.float64)
    for jj in range(8):
        out[:, jj:jj + 25] += w2
    return out / 8.0


_NC_CACHE = None


def kernel(query, key, value, wq, wk, wv, w_out, b_out):
    global _NC_CACHE, LAST_RESULTS
    query = np.ascontiguousarray(np.asarray(query, np.float32))
    key = np.ascontiguousarray(np.asarray(key, np.float32))
    value = np.ascontiguousarray(np.asarray(value, np.float32))

    wqf = _fold_pool_into_conv(wq)
    wkf = _fold_pool_into_conv(wk)
    wvf = _fold_pool_into_conv(wv) * float(np.asarray(w_out).reshape(()))
    wqT = np.ascontiguousarray(wqf.T.astype(np.float32))     # (32, 512)
    wkT = np.ascontiguousarray(wkf.T.astype(np.float32))
    wvT = np.ascontiguousarray(wvf.T.astype(np.float32))
    bb = np.asarray(b_out, np.float32).reshape(1, 1)

    if _NC_CACHE is None:
        _NC_CACHE = _build_nc()
    nc = _NC_CACHE

    in_maps = []
    for b in range(B):
        in_maps.append({
            "xq": np.ascontiguousarray(query[b, 0]),
            "xk": np.ascontiguousarray(key[b, 0]),
            "xv": np.ascontiguousarray(value[b, 0]),
            "wqT": wqT, "wkT": wkT, "wvT": wvT,
            "bout": bb,
        })

    res = run_bass_kernel_spmd(nc, in_maps, core_ids=list(range(B)),
                               trace=TRACE, **TRACE_KW)
    LAST_RESULTS = res
    x = np.stack([r["x_o"] for r in res.results])            # (B, S, C)
    attn = np.stack([r["attn_o"] for r in res.results])      # (B, NH, S, S)
    return x, attn


# revision 6
# speedup vs baseline: 1.1081x; 1.1081x over previous
"""Trainium2 Bass kernel for ConvMultiHeadAttention.

Reference computation (per batch element b):
  q/k/v projections: conv1d(25 taps along d_model=512, VALID) -> (512, 488, 16)
                     avgpool(8 along H) -> (512, 61, 16)
                     flat-reshape -> (S=976, 512) viewed as (S, 8 heads, 64)
  attention: scores = Q K^T / 8, attn = softmax, x = attn V
  outputs: x (B, 976, 512) scaled by w_out + b_out, and attn (B, 8, 976, 976)

Sharding: data-parallel over batch B=8, one batch element per NeuronCore.

Device algorithm per core:
  - conv+pool folded into a stride-8, width-32 conv == matmul W'(512,32) @ P(32,976)
    where P are strided patches of the input image (affine DMA gather).
  - conv output Y(512,976) is written flat to DRAM scratch; the reference's
    reshape (512*976,) -> (976, 512) is a free reinterpretation of that buffer.
  - Q,K are reloaded row-wise and PE-transposed into (d=64, s=976) per-head
    layout; V is used row-wise (s, d) directly.
  - pass A (per head, per 128-row q chunk): scores = QhT.T @ KhT in PSUM,
    ACT computes exp(s/8) with fused per-row accumulation (softmax denom),
    DVE normalizes, DMA writes attn rows.
  - pass B: transposed scores K-major, exp'd, then AV matmul accumulates
    outT(64, 976) over k chunks; PE-transpose + per-row scale gives x.

Matmul dtype: bf16 operands with fp32 PSUM accumulation (the fp32/f32r
matmul paths split the 4-byte weight load into two LW sub-instructions and
hit walrus's sync-wait-count limit under Tile). Scores feed a softmax over
~1000 near-uniform logits, so bf16 logit error is far below tolerance; the
single bf16 rounding of V/exp(s) costs ~1e-3 relative on x.
"""

import numpy as np

import concourse.bacc as bacc
import concourse.bass as bass
import concourse.tile as tile
from concourse import mybir
from concourse.bass_utils import run_bass_kernel_spmd
from concourse.masks import make_identity

F32 = mybir.dt.float32
BF16 = mybir.dt.float16  # fp16: same speed class as bf16, more mantissa
AF = mybir.ActivationFunctionType
ALU = mybir.AluOpType

B = 8
NH, DH = 8, 64
S = 976          # sequence positions after conv+pool reshape
C = 512          # channels = NH*DH
DM, L = 512, 16  # conv input "image": d_model x max_len
KW = 32          # folded conv width (25-tap conv + 8-wide avgpool, stride 8)
H2 = 61          # pooled positions, S = H2 * L

NSPLITS = ((0, 512), (512, S))                        # PSUM-bank splits of 976
SCH = [(i * 128, min(128, S - i * 128)) for i in range(8)]  # 7x128 + 80

TRACE = False
TRACE_KW: dict = {}
LAST_RESULTS = None


def _build_nc():
    nc = bacc.Bacc("TRN2", target_bir_lowering=False, debug=False)

    xin = {t: nc.dram_tensor(f"x{t}", [DM, L], F32, kind="ExternalInput")
           for t in "qkv"}
    win = {t: nc.dram_tensor(f"w{t}T", [KW, C], F32, kind="ExternalInput")
           for t in "qkv"}
    bout = nc.dram_tensor("bout", [1, 1], F32, kind="ExternalInput")
    attn_o = nc.dram_tensor("attn_o", [NH, S, S], F32, kind="ExternalOutput")
    x_o = nc.dram_tensor("x_o", [S, C], F32, kind="ExternalOutput")
    # conv outputs, stored flat in bf16
    y = {t: nc.dram_tensor(f"y{t}", [C * S], BF16, kind="Internal")
         for t in "qkv"}

    with tile.TileContext(nc) as tc:
        _body(tc, xin, win, bout, attn_o, x_o, y)
    nc.compile()
    return nc


def _body(tc, xin, win, bout, attn_o, x_o, y):
    nc = tc.nc
    import contextlib
    ctx = contextlib.ExitStack()
    with ctx:
        consts = ctx.enter_context(tc.tile_pool(name="consts", bufs=1))
        persist = ctx.enter_context(tc.tile_pool(name="persist", bufs=1))
        stage = ctx.enter_context(tc.tile_pool(name="stage", bufs=3))
        work = ctx.enter_context(tc.tile_pool(name="work", bufs=3))
        mm_psum = ctx.enter_context(tc.tile_pool(name="mm_psum", bufs=2, space="PSUM"))
        tp_psum = ctx.enter_context(tc.tile_pool(name="tp_psum", bufs=2, space="PSUM"))
        acc_psum = ctx.enter_context(tc.tile_pool(name="acc_psum", bufs=1, space="PSUM"))

        ident = consts.tile([128, 128], F32, tag="ident")
        make_identity(nc, ident)
        ident_h = consts.tile([128, 128], BF16, tag="ident_h")
        make_identity(nc, ident_h)

        bb = consts.tile([128, 1], F32, tag="bb")
        bsrc = bass.AP(tensor=bout.ap().tensor, offset=0, ap=[[0, 128], [1, 1]])
        nc.gpsimd.dma_start(out=bb, in_=bsrc)

        # ---- phase 1: conv projections -> DRAM scratch (k first: kT gates
        # the first scores matmul; v last: only needed at pass B) ----
        for t in "kqv":
            p_t = consts.tile([KW, H2, L], F32, tag=f"patch{t}", name=f"patch{t}")
            # P[k', h2, l] = x[8*h2 + k', l]
            psrc = bass.AP(tensor=xin[t].ap().tensor, offset=0,
                           ap=[[L, KW], [8 * L, H2], [1, L]])
            nc.sync.dma_start(out=p_t, in_=psrc)
            w_t = consts.tile([KW, C], F32, tag=f"w{t}", name=f"w{t}")
            nc.sync.dma_start(out=w_t, in_=win[t].ap())
            # round conv operands to bf16 for the fast matmul path
            p_r = consts.tile([KW, H2 * L], BF16, tag=f"pr{t}", name=f"pr{t}")
            nc.vector.tensor_copy(p_r, p_t.rearrange("k a b -> k (a b)"))
            w_r = consts.tile([KW, C], BF16, tag=f"wr{t}", name=f"wr{t}")
            nc.vector.tensor_copy(w_r, w_t)

            ywview = y[t].ap().rearrange("(c u) -> c u", u=S)
            for ci in range(4):
                ps = mm_psum.tile([128, S], F32, tag="mm")
                for (n0, n1) in NSPLITS:
                    nc.tensor.matmul(
                        ps[:, n0:n1],
                        lhsT=w_r[:, ci * 128:(ci + 1) * 128],
                        rhs=p_r[:, n0:n1],
                        start=True, stop=True)
                y_sb = stage.tile([128, S], BF16, tag="ysb")
                if ci % 2 == 0:
                    nc.scalar.copy(y_sb, ps)
                else:
                    nc.vector.tensor_copy(y_sb, ps)
                nc.sync.dma_start(out=ywview[ci * 128:(ci + 1) * 128, :], in_=y_sb)

        # ---- phase 2: per-head transposed Q,K (f32r); row-major V (f32r) ----
        qT = [persist.tile([128, S], BF16, tag=f"qT{j}", name=f"qT{j}") for j in range(4)]
        kT = [persist.tile([128, S], BF16, tag=f"kT{j}", name=f"kT{j}") for j in range(4)]
        rows_qk = {}
        for t, dst in (("k", kT), ("q", qT)):
            sview = y[t].ap().rearrange("(s d) -> s d", d=C)
            for sc, (s0, sz) in enumerate(SCH):
                rows = persist.tile([128, C], BF16, tag=f"rows{t}{sc}",
                                    name=f"rows{t}{sc}")
                rows_qk[(t, sc)] = rows
                nc.sync.dma_start(out=rows[:sz], in_=sview[s0:s0 + sz, :])
        v_rows = [persist.tile([128, C], BF16, tag=f"vr{sc}", name=f"vr{sc}")
                  for sc in range(8)]
        vview = y["v"].ap().rearrange("(s d) -> s d", d=C)
        for sc, (s0, sz) in enumerate(SCH):
            nc.sync.dma_start(out=v_rows[sc][:sz], in_=vview[s0:s0 + sz, :])
        # j=0 (heads 0,1) first so attention starts while j=1..3 transpose
        for j in (0, 1, 2, 3):
            for t, dst in (("k", kT), ("q", qT)):
                for sc, (s0, sz) in enumerate(SCH):
                    rows = rows_qk[(t, sc)]
                    pst = tp_psum.tile([128, 128], BF16, tag="tp")
                    nc.tensor.transpose(pst[:, :sz],
                                        rows[:sz, j * 128:(j + 1) * 128],
                                        ident_h[:sz, :sz])
                    nc.vector.tensor_copy(dst[j][:, s0:s0 + sz], pst[:, :sz])

        x_sb = [persist.tile([128, C], F32, tag=f"xsb{sc}", name=f"xsb{sc}")
                for sc in range(8)]

        # ---- phase 3: attention per head ----
        def pass_a(h, qTh, kTh, R_all):
            # q-major scores -> exp(+denom) -> normalize -> attn out
            for qc, (s0, sz) in enumerate(SCH):
                sA = mm_psum.tile([128, S], F32, tag="mm")
                for (n0, n1) in NSPLITS:
                    nc.tensor.matmul(sA[:sz, n0:n1],
                                     lhsT=qTh[:, s0:s0 + sz],
                                     rhs=kTh[:, n0:n1],
                                     start=True, stop=True)
                e = work.tile([128, S], F32, tag="e")
                d = work.tile([128, 1], F32, tag="d")
                nc.scalar.activation(e[:sz], sA[:sz], AF.Exp, scale=0.125,
                                     accum_out=d[:sz])
                nc.vector.reciprocal(R_all[:sz, qc:qc + 1], d[:sz])
                a = work.tile([128, S], F32, tag="a", bufs=4)
                nc.vector.tensor_scalar_mul(a[:sz], e[:sz], R_all[:sz, qc:qc + 1])
                nc.sync.dma_start(out=attn_o.ap()[h, s0:s0 + sz, :], in_=a[:sz])

        def pass_b(h, qTh, kTh):
            # k-major scores -> exp (fp16) -> AV accumulate (unnormalized)
            oT0 = acc_psum.tile([64, 512], F32, tag="oT0", name=f"oT0_{h}")
            oT1 = acc_psum.tile([64, S - 512], F32, tag="oT1", name=f"oT1_{h}")
            for kc, (s0, sz) in enumerate(SCH):
                sB = mm_psum.tile([128, S], F32, tag="mm")
                for (n0, n1) in NSPLITS:
                    nc.tensor.matmul(sB[:sz, n0:n1],
                                     lhsT=kTh[:, s0:s0 + sz],
                                     rhs=qTh[:, n0:n1],
                                     start=True, stop=True)
                eT = work.tile([128, S], BF16, tag="et")
                nc.scalar.activation(eT[:sz], sB[:sz], AF.Exp, scale=0.125)
                vh = v_rows[kc][:sz, h * DH:(h + 1) * DH]
                nc.tensor.matmul(oT0, lhsT=vh, rhs=eT[:sz, 0:512],
                                 start=(kc == 0), stop=(kc == 7))
                nc.tensor.matmul(oT1, lhsT=vh, rhs=eT[:sz, 512:S],
                                 start=(kc == 0), stop=(kc == 7))
            return oT0, oT1

        def out_phase(h, oT0, oT1, R_all):
            o_sb = work.tile([64, S], F32, tag="osb")
            nc.vector.tensor_copy(o_sb[:, 0:512], oT0)
            nc.vector.tensor_copy(o_sb[:, 512:S], oT1)
            for qc, (s0, sz) in enumerate(SCH):
                pst = tp_psum.tile([128, 128], F32, tag="tp")
                nc.tensor.transpose(pst[:sz, :DH], o_sb[:, s0:s0 + sz],
                                    ident[:64, :64])
                nc.vector.tensor_scalar(
                    out=x_sb[qc][:sz, h * DH:(h + 1) * DH],
                    in0=pst[:sz, :DH],
                    scalar1=R_all[:sz, qc:qc + 1],
                    scalar2=bb[:sz],
                    op0=ALU.mult, op1=ALU.add)

        for h in range(NH):
            jh, off = h // 2, (h % 2) * 64
            qTh = qT[jh][off:off + 64, :]
            kTh = kT[jh][off:off + 64, :]
            R_all = work.tile([128, 8], F32, tag="rall", name=f"rall{h}")
            if h < NH - 1:
                pass_a(h, qTh, kTh, R_all)
                oT0, oT1 = pass_b(h, qTh, kTh)
            else:
                # last head: B first so post-last-exp tail work is minimal
                oT0, oT1 = pass_b(h, qTh, kTh)
                pass_a(h, qTh, kTh, R_all)
            out_phase(h, oT0, oT1, R_all)

        for sc, (s0, sz) in enumerate(SCH):
            nc.sync.dma_start(out=x_o.ap()[s0:s0 + sz, :], in_=x_sb[sc][:sz])


def _fold_pool_into_conv(w):
    """(C,1,25,1) conv weight + 8-wide avgpool -> (C, 32) stride-8 weights."""
    w2 = np.asarray(w, np.float64)[:, 0, :, 0]          # (C, 25)
    out = np.zeros((w2.shape[0], KW), np.float64)
    for jj in range(8):
        out[:, jj:jj + 25] += w2
    return out / 8.0


_NC_CACHE = None


def kernel(query, key, value, wq, wk, wv, w_out, b_out):
    global _NC_CACHE, LAST_RESULTS
    query = np.ascontiguousarray(np.asarray(query, np.float32))
    key = np.ascontiguousarray(np.asarray(key, np.float32))
    value = np.ascontiguousarray(np.asarray(value, np.float32))

    wqf = _fold_pool_into_conv(wq)
    wkf = _fold_pool_into_conv(wk)
    wvf = _fold_pool_into_conv(wv) * float(np.asarray(w_out).reshape(()))
    wqT = np.ascontiguousarray(wqf.T.astype(np.float32))     # (32, 512)
    wkT = np.ascontiguousarray(wkf.T.astype(np.float32))
    wvT = np.ascontiguousarray(wvf.T.astype(np.float32))
    bb = np.asarray(b_out, np.float32).reshape(1, 1)

    if _NC_CACHE is None:
        _NC_CACHE = _build_nc()
    nc = _NC_CACHE

    in_maps = []
    for b in range(B):
        in_maps.append({
            "xq": np.ascontiguousarray(query[b, 0]),
            "xk": np.ascontiguousarray(key[b, 0]),
            "xv": np.ascontiguousarray(value[b, 0]),
            "wqT": wqT, "wkT": wkT, "wvT": wvT,
            "bout": bb,
        })

    res = run_bass_kernel_spmd(nc, in_maps, core_ids=list(range(B)),
                               trace=TRACE, **TRACE_KW)
    LAST_RESULTS = res
    x = np.stack([r["x_o"] for r in res.results])            # (B, S, C)
    attn = np.stack([r["attn_o"] for r in res.results])      # (B, NH, S, S)
    return x, attn


# revision 8
# speedup vs baseline: 1.1520x; 1.0396x over previous
"""Trainium2 Bass kernel for ConvMultiHeadAttention.

Reference computation (per batch element b):
  q/k/v projections: conv1d(25 taps along d_model=512, VALID) -> (512, 488, 16)
                     avgpool(8 along H) -> (512, 61, 16)
                     flat-reshape -> (S=976, 512) viewed as (S, 8 heads, 64)
  attention: scores = Q K^T / 8, attn = softmax, x = attn V
  outputs: x (B, 976, 512) scaled by w_out + b_out, and attn (B, 8, 976, 976)

Sharding: data-parallel over batch B=8, one batch element per NeuronCore.

Device algorithm per core:
  - conv+pool folded into a stride-8, width-32 conv == matmul W'(512,32) @ P(32,976)
    where P are strided patches of the input image (affine DMA gather).
  - conv output Y(512,976) is written flat to DRAM scratch; the reference's
    reshape (512*976,) -> (976, 512) is a free reinterpretation of that buffer.
  - Q,K are reloaded row-wise and PE-transposed into (d=64, s=976) per-head
    layout; V is used row-wise (s, d) directly.
  - pass A (per head, per 128-row q chunk): scores = QhT.T @ KhT in PSUM,
    ACT computes exp(s/8) with fused per-row accumulation (softmax denom),
    DVE normalizes, DMA writes attn rows.
  - pass B: transposed scores K-major, exp'd, then AV matmul accumulates
    outT(64, 976) over k chunks; PE-transpose + per-row scale gives x.

Matmul dtype: bf16 operands with fp32 PSUM accumulation (the fp32/f32r
matmul paths split the 4-byte weight load into two LW sub-instructions and
hit walrus's sync-wait-count limit under Tile). Scores feed a softmax over
~1000 near-uniform logits, so bf16 logit error is far below tolerance; the
single bf16 rounding of V/exp(s) costs ~1e-3 relative on x.
"""

import numpy as np

import concourse.bacc as bacc
import concourse.bass as bass
import concourse.tile as tile
from concourse import mybir
from concourse.bass_utils import run_bass_kernel_spmd
from concourse.masks import make_identity

F32 = mybir.dt.float32
BF16 = mybir.dt.float16  # fp16: same speed class as bf16, more mantissa
AF = mybir.ActivationFunctionType
ALU = mybir.AluOpType

B = 8
NH, DH = 8, 64
S = 976          # sequence positions after conv+pool reshape
C = 512          # channels = NH*DH
DM, L = 512, 16  # conv input "image": d_model x max_len
KW = 32          # folded conv width (25-tap conv + 8-wide avgpool, stride 8)
H2 = 61          # pooled positions, S = H2 * L

NSPLITS = ((0, 512), (512, S))                        # PSUM-bank splits of 976
SCH = [(i * 128, min(128, S - i * 128)) for i in range(8)]  # 7x128 + 80

TRACE = False
TRACE_KW: dict = {}
LAST_RESULTS = None


def _build_nc():
    nc = bacc.Bacc("TRN2", target_bir_lowering=False, debug=False)

    xin = {t: nc.dram_tensor(f"x{t}", [DM, L], F32, kind="ExternalInput")
           for t in "qkv"}
    win = {t: nc.dram_tensor(f"w{t}T", [KW, C], F32, kind="ExternalInput")
           for t in "qkv"}
    bout = nc.dram_tensor("bout", [1, 1], F32, kind="ExternalInput")
    attn_o = nc.dram_tensor("attn_o", [NH, S, S], F32, kind="ExternalOutput")
    x_o = nc.dram_tensor("x_o", [S, C], F32, kind="ExternalOutput")
    # conv outputs, stored flat in fp16; the reference's reshape
    # (512*976,) -> (976, 512) is a free reinterpretation of this buffer
    y = {t: nc.dram_tensor(f"y{t}", [C * S], BF16, kind="Internal")
         for t in "qkv"}

    with tile.TileContext(nc) as tc:
        _body(tc, xin, win, bout, attn_o, x_o, y)
    nc.compile()
    return nc


def _body(tc, xin, win, bout, attn_o, x_o, y):
    nc = tc.nc
    import contextlib
    ctx = contextlib.ExitStack()
    with ctx:
        consts = ctx.enter_context(tc.tile_pool(name="consts", bufs=1))
        persist = ctx.enter_context(tc.tile_pool(name="persist", bufs=1))
        stage = ctx.enter_context(tc.tile_pool(name="stage", bufs=3))
        work = ctx.enter_context(tc.tile_pool(name="work", bufs=3))
        mm_psum = ctx.enter_context(tc.tile_pool(name="mm_psum", bufs=2, space="PSUM"))
        tp_psum = ctx.enter_context(tc.tile_pool(name="tp_psum", bufs=2, space="PSUM"))
        acc_psum = ctx.enter_context(tc.tile_pool(name="acc_psum", bufs=1, space="PSUM"))

        ident = consts.tile([128, 128], F32, tag="ident")
        make_identity(nc, ident)
        ident_h = consts.tile([128, 128], BF16, tag="ident_h")
        make_identity(nc, ident_h)

        bb = consts.tile([128, 1], F32, tag="bb")
        bsrc = bass.AP(tensor=bout.ap().tensor, offset=0, ap=[[0, 128], [1, 1]])
        nc.gpsimd.dma_start(out=bb, in_=bsrc)

        # ---- phase 1: conv projections -> DRAM scratch (k first: kT gates
        # the first scores matmul; v last: only needed at pass B) ----
        for t in "kqv":
            p_t = consts.tile([KW, H2, L], F32, tag=f"patch{t}", name=f"patch{t}")
            # P[k', h2, l] = x[8*h2 + k', l]
            psrc = bass.AP(tensor=xin[t].ap().tensor, offset=0,
                           ap=[[L, KW], [8 * L, H2], [1, L]])
            nc.sync.dma_start(out=p_t, in_=psrc)
            w_t = consts.tile([KW, C], F32, tag=f"w{t}", name=f"w{t}")
            nc.sync.dma_start(out=w_t, in_=win[t].ap())
            # round conv operands to bf16 for the fast matmul path
            p_r = consts.tile([KW, H2 * L], BF16, tag=f"pr{t}", name=f"pr{t}")
            nc.vector.tensor_copy(p_r, p_t.rearrange("k a b -> k (a b)"))
            w_r = consts.tile([KW, C], BF16, tag=f"wr{t}", name=f"wr{t}")
            nc.vector.tensor_copy(w_r, w_t)

            ywview = y[t].ap().rearrange("(c u) -> c u", u=S)
            for ci in range(4):
                ps = mm_psum.tile([128, S], F32, tag="mm")
                for (n0, n1) in NSPLITS:
                    nc.tensor.matmul(
                        ps[:, n0:n1],
                        lhsT=w_r[:, ci * 128:(ci + 1) * 128],
                        rhs=p_r[:, n0:n1],
                        start=True, stop=True)
                y_sb = stage.tile([128, S], BF16, tag="ysb")
                if ci % 2 == 0:
                    nc.scalar.copy(y_sb, ps)
                else:
                    nc.vector.tensor_copy(y_sb, ps)
                nc.sync.dma_start(out=ywview[ci * 128:(ci + 1) * 128, :], in_=y_sb)

        # ---- phase 2: per-head transposed Q,K (f32r); row-major V (f32r) ----
        qT = [persist.tile([128, S], BF16, tag=f"qT{j}", name=f"qT{j}") for j in range(4)]
        kT = [persist.tile([128, S], BF16, tag=f"kT{j}", name=f"kT{j}") for j in range(4)]
        rows_qk = {}
        for t, dst in (("k", kT), ("q", qT)):
            sview = y[t].ap().rearrange("(s d) -> s d", d=C)
            for sc, (s0, sz) in enumerate(SCH):
                rows = persist.tile([128, C], BF16, tag=f"rows{t}{sc}",
                                    name=f"rows{t}{sc}")
                rows_qk[(t, sc)] = rows
                nc.sync.dma_start(out=rows[:sz], in_=sview[s0:s0 + sz, :])
        v_rows = [persist.tile([128, C], BF16, tag=f"vr{sc}", name=f"vr{sc}")
                  for sc in range(8)]
        vview = y["v"].ap().rearrange("(s d) -> s d", d=C)
        for sc, (s0, sz) in enumerate(SCH):
            nc.sync.dma_start(out=v_rows[sc][:sz], in_=vview[s0:s0 + sz, :])
        # j=0 (heads 0,1) first so attention starts while j=1..3 transpose
        for j in (0, 1, 2, 3):
            for t, dst in (("k", kT), ("q", qT)):
                for sc, (s0, sz) in enumerate(SCH):
                    rows = rows_qk[(t, sc)]
                    pst = tp_psum.tile([128, 128], BF16, tag="tp")
                    nc.tensor.transpose(pst[:, :sz],
                                        rows[:sz, j * 128:(j + 1) * 128],
                                        ident_h[:sz, :sz])
                    nc.vector.tensor_copy(dst[j][:, s0:s0 + sz], pst[:, :sz])

        x_sb = [persist.tile([128, C], F32, tag=f"xsb{sc}", name=f"xsb{sc}")
                for sc in range(8)]

        # ---- phase 3: attention per head ----
        def pass_a(h, qTh, kTh, R_all):
            # q-major scores -> exp(+denom) -> normalize -> attn out
            for qc, (s0, sz) in enumerate(SCH):
                sA = mm_psum.tile([128, S], F32, tag="mm")
                for (n0, n1) in NSPLITS:
                    nc.tensor.matmul(sA[:sz, n0:n1],
                                     lhsT=qTh[:, s0:s0 + sz],
                                     rhs=kTh[:, n0:n1],
                                     start=True, stop=True)
                e = work.tile([128, S], F32, tag="e")
                d = work.tile([128, 1], F32, tag="d")
                nc.scalar.activation(e[:sz], sA[:sz], AF.Exp, scale=0.125,
                                     accum_out=d[:sz])
                nc.vector.reciprocal(R_all[:sz, qc:qc + 1], d[:sz])
                a = work.tile([128, S], F32, tag="a", bufs=4)
                nc.vector.tensor_scalar_mul(a[:sz], e[:sz], R_all[:sz, qc:qc + 1])
                nc.sync.dma_start(out=attn_o.ap()[h, s0:s0 + sz, :], in_=a[:sz])

        def pass_b(h, qTh, kTh):
            # k-major scores -> exp (fp16) -> AV accumulate (unnormalized)
            oT0 = acc_psum.tile([64, 512], F32, tag="oT0", name=f"oT0_{h}")
            oT1 = acc_psum.tile([64, S - 512], F32, tag="oT1", name=f"oT1_{h}")
            for kc, (s0, sz) in enumerate(SCH):
                sB = mm_psum.tile([128, S], F32, tag="mm")
                for (n0, n1) in NSPLITS:
                    nc.tensor.matmul(sB[:sz, n0:n1],
                                     lhsT=kTh[:, s0:s0 + sz],
                                     rhs=qTh[:, n0:n1],
                                     start=True, stop=True)
                eT = work.tile([128, S], BF16, tag="et")
                nc.scalar.activation(eT[:sz], sB[:sz], AF.Exp, scale=0.125)
                vh = v_rows[kc][:sz, h * DH:(h + 1) * DH]
                nc.tensor.matmul(oT0, lhsT=vh, rhs=eT[:sz, 0:512],
                                 start=(kc == 0), stop=(kc == 7))
                nc.tensor.matmul(oT1, lhsT=vh, rhs=eT[:sz, 512:S],
                                 start=(kc == 0), stop=(kc == 7))
            return oT0, oT1

        def out_phase(h, oT0, oT1, R_all):
            o_sb = work.tile([64, S], F32, tag="osb")
            nc.vector.tensor_copy(o_sb[:, 0:512], oT0)
            nc.vector.tensor_copy(o_sb[:, 512:S], oT1)
            for qc, (s0, sz) in enumerate(SCH):
                pst = tp_psum.tile([128, 128], F32, tag="tp")
                nc.tensor.transpose(pst[:sz, :DH], o_sb[:, s0:s0 + sz],
                                    ident[:64, :64])
                nc.vector.tensor_scalar(
                    out=x_sb[qc][:sz, h * DH:(h + 1) * DH],
                    in0=pst[:sz, :DH],
                    scalar1=R_all[:sz, qc:qc + 1],
                    scalar2=bb[:sz],
                    op0=ALU.mult, op1=ALU.add)

        for h in range(NH):
            jh, off = h // 2, (h % 2) * 64
            qTh = qT[jh][off:off + 64, :]
            kTh = kT[jh][off:off + 64, :]
            R_all = work.tile([128, 8], F32, tag="rall", name=f"rall{h}")
            if h < NH - 1:
                pass_a(h, qTh, kTh, R_all)
                oT0, oT1 = pass_b(h, qTh, kTh)
            else:
                # last head: B first so post-last-exp tail work is minimal
                oT0, oT1 = pass_b(h, qTh, kTh)
                pass_a(h, qTh, kTh, R_all)
            out_phase(h, oT0, oT1, R_all)

        for sc, (s0, sz) in enumerate(SCH):
            nc.sync.dma_start(out=x_o.ap()[s0:s0 + sz, :], in_=x_sb[sc][:sz])


def _fold_pool_into_conv(w):
    """(C,1,25,1) conv weight + 8-wide avgpool -> (C, 32) stride-8 weights."""
    w2 = np.asarray(w, np.float64)[:, 0, :, 0]          # (C, 25)
    out = np.zeros((w2.shape[0], KW), np.float64)
    for jj in range(8):
        out[:, jj:jj + 25] += w2
    return out / 8.0


_NC_CACHE = None


def kernel(query, key, value, wq, wk, wv, w_out, b_out):
    global _NC_CACHE, LAST_RESULTS
    query = np.ascontiguousarray(np.asarray(query, np.float32))
    key = np.ascontiguousarray(np.asarray(key, np.float32))
    value = np.ascontiguousarray(np.asarray(value, np.float32))

    wqf = _fold_pool_into_conv(wq)
    wkf = _fold_pool_into_conv(wk)
    wvf = _fold_pool_into_conv(wv) * float(np.asarray(w_out).reshape(()))
    wqT = np.ascontiguousarray(wqf.T.astype(np.float32))     # (32, 512)
    wkT = np.ascontiguousarray(wkf.T.astype(np.float32))
    wvT = np.ascontiguousarray(wvf.T.astype(np.float32))
    bb = np.asarray(b_out, np.float32).reshape(1, 1)

    if _NC_CACHE is None:
        _NC_CACHE = _build_nc()
    nc = _NC_CACHE

    in_maps = []
    for b in range(B):
        in_maps.append({
            "xq": np.ascontiguousarray(query[b, 0]),
            "xk": np.ascontiguousarray(key[b, 0]),
            "xv": np.ascontiguousarray(value[b, 0]),
            "wqT": wqT, "wkT": wkT, "wvT": wvT,
            "bout": bb,
        })

    res = run_bass_kernel_spmd(nc, in_maps, core_ids=list(range(B)),
                               trace=TRACE, **TRACE_KW)
    LAST_RESULTS = res
    x = np.stack([r["x_o"] for r in res.results])            # (B, S, C)
    attn = np.stack([r["attn_o"] for r in res.results])      # (B, NH, S, S)
    return x, attn
